# revision 21
# baseline (speedup 1.0000x reference)
"""Trainium2 Bass kernel for nn_Attention_9431748182617.

Quirky attention: scores z[b,k,q] = (q_h . k_h) / sqrt(D), softmax over the
QUERY axis (per key row), out[q] = sum_k A[k,q] * v[k], then output projection.

Sharding (8 NeuronCores):
  - tensor-parallel over heads: 16 heads -> 2 heads per core.
    Each core owns rows [128c, 128c+128) of Wq/Wk/Wv (its 2 heads) and
    computes q/k/v + attention for those heads over the full batch.
  - z^T (local 128 rows of L, all of B*S) is AllGather'd per batch.
  - output projection sharded by output feature D: core c computes
    out^T rows [128c, 128c+128) using Wo^T[:, 128c:128c+128] for ALL s.
  - host concatenates the 8 out^T blocks and transposes.

All matmuls in bf16 (fp32 PSUM accumulation); exp on ScalarE in fp32 with
fused free-axis accumulation for softmax denominators; 1/denom is folded
into V rows (per-partition scalar) so no full-size normalization pass.
"""

import os

import numpy as np
import ml_dtypes

import concourse.bass as bass
import concourse.mybir as mybir
import concourse.tile as tile
from concourse.bass_utils import run_bass_kernel_spmd
from concourse.masks import make_identity

B, S, D = 4, 2048, 1024
L, H = 1024, 16
DH = L // H               # 64
NCORES = 8
LPC = L // NCORES         # 128 l-rows (= 2 heads) per core
DPC = D // NCORES         # 128 out-feature rows per core
SCALE = 1.0 / (D ** 0.5)
KC = S // 128             # 16 key chunks of 128
BF16 = mybir.dt.bfloat16
F32 = mybir.dt.float32
EXP = mybir.ActivationFunctionType.Exp

LAST_EXEC_NS = None


def _body(tc, xT, wqT, wkT, wvT, woT, outT, zloc, zfull):
    nc = tc.nc
    from contextlib import ExitStack

    with ExitStack() as ctx:
        const = ctx.enter_context(tc.tile_pool(name="const", bufs=1))
        xpool = ctx.enter_context(tc.tile_pool(name="xpool", bufs=2))
        qk = ctx.enter_context(tc.tile_pool(name="qk", bufs=2))
        vtpool = ctx.enter_context(tc.tile_pool(name="vtpool", bufs=1))
        vpool = ctx.enter_context(tc.tile_pool(name="vpool", bufs=2))
        apool = ctx.enter_context(tc.tile_pool(name="apool", bufs=2))
        small = ctx.enter_context(tc.tile_pool(name="small", bufs=4))
        ztp = ctx.enter_context(tc.tile_pool(name="ztp", bufs=1))
        zslab = ctx.enter_context(tc.tile_pool(name="zslab", bufs=2))
        osb_p = ctx.enter_context(tc.tile_pool(name="osb_p", bufs=1))
        ps = ctx.enter_context(tc.tile_pool(name="ps", bufs=1, space="PSUM"))

        # ---- constants: weights + identity ----
        wq_sb = const.tile([128, 8, 128], BF16, name="wq_sb")
        wk_sb = const.tile([128, 8, 128], BF16, name="wk_sb")
        wv_sb = const.tile([128, 8, 128], BF16, name="wv_sb")
        wo_sb = const.tile([128, 8, 128], BF16, name="wo_sb")
        for dc in range(8):
            nc.sync.dma_start(wq_sb[:, dc, :], wqT[dc * 128:(dc + 1) * 128, :])
            nc.sync.dma_start(wk_sb[:, dc, :], wkT[dc * 128:(dc + 1) * 128, :])
            nc.sync.dma_start(wv_sb[:, dc, :], wvT[dc * 128:(dc + 1) * 128, :])
            nc.sync.dma_start(wo_sb[:, dc, :], woT[dc * 128:(dc + 1) * 128, :])
        ident = const.tile([128, 128], BF16, name="ident")
        make_identity(nc, ident)

        def proj(b):
            """QKV projections for batch b -> (qt, kt, v) sbuf tiles."""
            x_c = []
            for dc in range(8):
                xc = xpool.tile([128, S], BF16, name=f"xc{dc}", tag=f"x{dc}")
                nc.gpsimd.dma_start(xc, xT[b, dc * 128:(dc + 1) * 128, :])
                x_c.append(xc)

            outs = []
            for w_sb, nm in ((wq_sb, "qt"), (wk_sb, "kt"), (wv_sb, "vt")):
                pool = vtpool if nm == "vt" else qk
                dest = pool.tile([128, S], BF16, name=nm, tag=nm)
                for sc in range(4):
                    pw = ps.tile([128, 512], F32, name="pw", tag="work")
                    for dc in range(8):
                        nc.tensor.matmul(
                            pw,
                            lhsT=w_sb[:, dc, :],
                            rhs=x_c[dc][:, sc * 512:(sc + 1) * 512],
                            start=(dc == 0),
                            stop=(dc == 7),
                        )
                    nc.vector.tensor_copy(dest[:, sc * 512:(sc + 1) * 512], pw)
                outs.append(dest)
            qt, kt, vt = outs

            # transpose VT [dh2, s] -> V [s, dh2] in 128-chunks
            v_sb = vpool.tile([128, KC, 128], BF16, name="v_sb", tag="v")
            for c in range(KC):
                pt = ps.tile([128, 128], BF16, name="pt", tag="work")
                nc.tensor.transpose(pt, vt[:, c * 128:(c + 1) * 128], ident)
                nc.vector.tensor_copy(v_sb[:, c, :], pt)
            return qt, kt, v_sb

        def scores_exp(kc, qt, kt, v_sb):
            """Scores + exp + denominators + scaled V for key-chunk kc."""
            res = []
            for h in range(2):
                hp = h * 64
                a_t = apool.tile([128, S], BF16, name=f"a{h}", tag=f"a{h}")
                accs = []
                for half in range(2):
                    pw = ps.tile([128, 1024], F32, name="psc", tag="work")
                    for qq in range(2):
                        q0 = half * 1024 + qq * 512
                        nc.tensor.matmul(
                            pw[:, qq * 512:(qq + 1) * 512],
                            lhsT=kt[hp:hp + 64, kc * 128:(kc + 1) * 128],
                            rhs=qt[hp:hp + 64, q0:q0 + 512],
                            start=True,
                            stop=True,
                        )
                    acc = small.tile([128, 1], F32, name="acc", tag="acc")
                    nc.scalar.activation(
                        a_t[:, half * 1024:(half + 1) * 1024],
                        pw,
                        EXP,
                        scale=float(SCALE),
                        accum_out=acc,
                    )
                    accs.append(acc)
                den = small.tile([128, 1], F32, name="den", tag="den")
                nc.vector.tensor_add(den, accs[0], accs[1])
                rec = small.tile([128, 1], F32, name="rec", tag="rec")
                nc.vector.reciprocal(rec, den)
                vs = small.tile([128, DH], BF16, name="vs", tag=f"vs{h}")
                nc.vector.tensor_scalar_mul(vs, v_sb[:, kc, hp:hp + 64], rec)
                res.append((a_t, vs))
            return res

        def av(kc, pair, zt_ps):
            """z^T += Vs^T @ A for key-chunk kc (both heads, col-tiled)."""
            for h in range(2):
                a_t, vs = pair[h]
                hp = h * 64
                for qc in range(4):
                    nc.tensor.matmul(
                        zt_ps[hp:hp + 64, qc * 512:(qc + 1) * 512],
                        lhsT=vs,
                        rhs=a_t[:, qc * 512:(qc + 1) * 512],
                        start=(kc == 0),
                        stop=(kc == KC - 1),
                        skip_group_check=True,
                    )

        def attention(b, qt, kt, v_sb):
            zt_ps = ps.tile([128, S], F32, name="zt_ps", tag="zt")
            prev = None
            for kc in range(KC):
                cur = (kc, scores_exp(kc, qt, kt, v_sb))
                if prev is not None:
                    av(prev[0], prev[1], zt_ps)
                prev = cur
            av(prev[0], prev[1], zt_ps)

            zt_sb = ztp.tile([128, S], BF16, name="zt_sb", tag="zt_sb")
            for hh in range(2):
                nc.vector.tensor_copy(
                    zt_sb[:, hh * 1024:(hh + 1) * 1024],
                    zt_ps[:, hh * 1024:(hh + 1) * 1024],
                )
            nc.sync.dma_start(zloc[b], zt_sb)
            nc.gpsimd.collective_compute(
                "AllGather",
                mybir.AluOpType.bypass,
                replica_groups=[list(range(NCORES))],
                ins=[zloc[b].opt()],
                outs=[zfull[b][:, :].opt()],
            )

        def outproj(b):
            zf_c = []
            for j in range(4):
                zf = zslab.tile([128, 2, S], BF16, name=f"zf{j}", tag=f"zf{j}")
                nc.gpsimd.dma_start(
                    zf,
                    zfull[b][j * 256:(j + 1) * 256, :].rearrange(
                        "(c p) s -> p c s", p=128
                    ),
                )
                zf_c.append(zf)
            o_sb = osb_p.tile([128, S], F32, name="o_sb", tag="osb")
            for sc in range(4):
                pw = ps.tile([128, 512], F32, name="po", tag="work")
                for lc in range(8):
                    nc.tensor.matmul(
                        pw,
                        lhsT=wo_sb[:, lc, :],
                        rhs=zf_c[lc // 2][:, lc % 2, sc * 512:(sc + 1) * 512],
                        start=(lc == 0),
                        stop=(lc == 7),
                    )
                nc.vector.tensor_copy(o_sb[:, sc * 512:(sc + 1) * 512], pw)
            nc.sync.dma_start(outT[:, b * S:(b + 1) * S], o_sb)

        for b in range(B):
            qt, kt, v_sb = proj(b)
            attention(b, qt, kt, v_sb)
            if b >= 1:
                outproj(b - 1)
        outproj(B - 1)


def _legalize_waits(nc):
    """This walrus build accepts only ~2 sync commands (1 wait + 1 inc) per
    instruction for the standard engine/DMA templates; Tile can emit 2-3
    waits (WAR + WAW + RAW). Hoist all but one wait of any multi-wait
    instruction onto single-wait NOPs on the same engine, immediately
    before it — the raw-bass `wait_ge; op` pattern. Drain/EventSemaphore
    templates accept many waits (the kernel-tail barrier relies on it)."""
    import bass_rust

    n = 0
    for f in nc.m.functions:
        for blk in f.blocks:
            out = []
            changed = False
            for inst in blk.instructions:
                si = inst.sync_info
                if si is not None and len(si.on_wait) > 1:
                    for w in si.on_wait[:-1]:
                        n += 1
                        out.append(
                            bass_rust.InstNoOp(
                                name=f"I-hoistwait-{n}",
                                engine=inst.engine,
                                bass_nofuse=True,
                                sync_info=bass_rust.SyncInfo(
                                    on_wait=[w], on_update=[]
                                ),
                            )
                        )
                    inst.sync_info = bass_rust.SyncInfo(
                        on_wait=[si.on_wait[-1]], on_update=list(si.on_update)
                    )
                    changed = True
                out.append(inst)
            if changed:
                blk.instructions = out


def build():
    nc = bass.Bass(
        "TRN2",
        target_bir_lowering=False,
        debug=False,
        enable_asserts=False,
        num_devices=NCORES,
    )
    xT = nc.dram_tensor("xT", [B, D, S], BF16, kind="ExternalInput").ap()
    wqT = nc.dram_tensor("wqT", [D, LPC], BF16, kind="ExternalInput").ap()
    wkT = nc.dram_tensor("wkT", [D, LPC], BF16, kind="ExternalInput").ap()
    wvT = nc.dram_tensor("wvT", [D, LPC], BF16, kind="ExternalInput").ap()
    woT = nc.dram_tensor("woT", [L, DPC], BF16, kind="ExternalInput").ap()
    outT = nc.dram_tensor("outT", [DPC, B * S], F32, kind="ExternalOutput").ap()

    with tile.TileContext(nc) as tc:
        from contextlib import ExitStack

        with ExitStack() as ctx:
            dram = ctx.enter_context(tc.tile_pool(name="dram", bufs=1, space="DRAM"))
            zloc = dram.tile([B, LPC, S], BF16, name="zloc")
            zfull = [
                dram.tile([L, S], BF16, name=f"zfull{b}", tag=f"zfull{b}",
                          addr_space="Shared")
                for b in range(B)
            ]
            _body(tc, xT, wqT, wkT, wvT, woT, outT, zloc, zfull)
    _legalize_waits(nc)
    return nc


def make_in_maps(x, Wq, Wk, Wv, Wo):
    bf = ml_dtypes.bfloat16
    x = np.asarray(x, np.float32)
    xT = np.ascontiguousarray(x.transpose(0, 2, 1)).astype(bf)  # (B, D, S)
    WoT = np.ascontiguousarray(np.asarray(Wo, np.float32).T)    # (L, D)
    in_maps = []
    for c in range(NCORES):
        rs = slice(128 * c, 128 * (c + 1))
        in_maps.append({
            "xT": xT,
            "wqT": np.ascontiguousarray(np.asarray(Wq, np.float32)[rs].T).astype(bf),
            "wkT": np.ascontiguousarray(np.asarray(Wk, np.float32)[rs].T).astype(bf),
            "wvT": np.ascontiguousarray(np.asarray(Wv, np.float32)[rs].T).astype(bf),
            "woT": np.ascontiguousarray(WoT[:, rs]).astype(bf),
        })
    return in_maps


def _install_ntff_hook_shim():
    """This container's `antenv` lacks `axon_hooks`; recreate the NTFF
    profile hook (same ctypes recipe as trn_agent_boot.trn_boot) so
    run_bass_kernel_spmd(trace=True) can capture exec_time_ns."""
    import sys
    import types
    import ctypes
    import contextlib

    try:
        import antenv.axon_hooks  # noqa: F401
        return
    except ImportError:
        pass

    hook = None
    so_path = os.environ.get("PJRT_LIBRARY_PATH")
    if so_path and os.path.exists(so_path):
        try:
            lib = ctypes.CDLL(so_path)
            if hasattr(lib, "axon_start_nrt_profile"):
                lib.axon_start_nrt_profile.argtypes = [
                    ctypes.POINTER(ctypes.c_int64),
                    ctypes.c_size_t,
                ]
                lib.axon_start_nrt_profile.restype = ctypes.c_int64
                lib.axon_stop_nrt_profile.argtypes = [ctypes.c_char_p]
                lib.axon_stop_nrt_profile.restype = ctypes.c_int64

                @contextlib.contextmanager
                def _hook(output_dir, device_ids):
                    import jax

                    jax.devices()
                    if device_ids:
                        ids = (ctypes.c_int64 * len(device_ids))(*device_ids)
                        rc = lib.axon_start_nrt_profile(ids, len(device_ids))
                    else:
                        rc = lib.axon_start_nrt_profile(None, 0)
                    if rc != 0:
                        raise RuntimeError(f"axon_start_nrt_profile rc={rc}")
                    try:
                        yield
                    finally:
                        n = lib.axon_stop_nrt_profile(str(output_dir).encode())
                        print(f"profile: {n} file(s) written to {output_dir}")

                hook = _hook
        except OSError:
            hook = None

    mod = types.ModuleType("antenv.axon_hooks")
    mod.get_axon_ntff_profile_hook = lambda: hook
    mod.set_axon_ntff_profile_hook = lambda h: None
    sys.modules["antenv.axon_hooks"] = mod
    import antenv

    antenv.axon_hooks = mod


def kernel(x, Wq, Wk, Wv, Wo):
    global LAST_EXEC_NS
    in_maps = make_in_maps(x, Wq, Wk, Wv, Wo)
    nc = build()
    trace = bool(int(os.environ.get("BASS_KERNEL_TRACE", "0")))
    if trace:
        _install_ntff_hook_shim()
    res = run_bass_kernel_spmd(nc, in_maps, core_ids=list(range(NCORES)), trace=trace)
    LAST_EXEC_NS = res.exec_time_ns
    outT = np.concatenate(
        [np.asarray(res.results[c]["outT"], np.float32) for c in range(NCORES)], axis=0
    )  # (D, B*S)
    return np.ascontiguousarray(outT.T).reshape(B, S, D).astype(np.float32)


# revision 25
# speedup vs baseline: 1.0325x; 1.0325x over previous
"""Trainium2 Bass kernel for nn_Attention_9431748182617.

Quirky attention: scores z[b,k,q] = (q_h . k_h) / sqrt(D), softmax over the
QUERY axis (per key row), out[q] = sum_k A[k,q] * v[k], then output projection.

Sharding (8 NeuronCores):
  - tensor-parallel over heads: 16 heads -> 2 heads per core.
    Each core owns rows [128c, 128c+128) of Wq/Wk/Wv (its 2 heads) and
    computes q/k/v + attention for those heads over the full batch.
  - z^T (local 128 rows of L, all of B*S) is AllGather'd per batch.
  - output projection sharded by output feature D: core c computes
    out^T rows [128c, 128c+128) using Wo^T[:, 128c:128c+128] for ALL s.
  - host concatenates the 8 out^T blocks and transposes.

All matmuls in bf16 (fp32 PSUM accumulation); exp on ScalarE in fp32 with
fused free-axis accumulation for softmax denominators; 1/denom is folded
into V rows (per-partition scalar) so no full-size normalization pass.
"""

import os

import numpy as np
import ml_dtypes

import concourse.bass as bass
import concourse.mybir as mybir
import concourse.tile as tile
from concourse.bass_utils import run_bass_kernel_spmd
from concourse.masks import make_identity

B, S, D = 4, 2048, 1024
L, H = 1024, 16
DH = L // H               # 64
NCORES = 8
LPC = L // NCORES         # 128 l-rows (= 2 heads) per core
DPC = D // NCORES         # 128 out-feature rows per core
SCALE = 1.0 / (D ** 0.5)
KC = S // 128             # 16 key chunks of 128
BF16 = mybir.dt.bfloat16
F32 = mybir.dt.float32
EXP = mybir.ActivationFunctionType.Exp

LAST_EXEC_NS = None


def _body(tc, xT, wqT, wkT, wvT, woT, outT, zloc, zfull):
    nc = tc.nc
    from contextlib import ExitStack

    with ExitStack() as ctx:
        const = ctx.enter_context(tc.tile_pool(name="const", bufs=1))
        xpool = ctx.enter_context(tc.tile_pool(name="xpool", bufs=1))
        qk = ctx.enter_context(tc.tile_pool(name="qk", bufs=2))
        vtpool = ctx.enter_context(tc.tile_pool(name="vtpool", bufs=1))
        vpool = ctx.enter_context(tc.tile_pool(name="vpool", bufs=2))
        apool = ctx.enter_context(tc.tile_pool(name="apool", bufs=5))
        small = ctx.enter_context(tc.tile_pool(name="small", bufs=6))
        ztp = ctx.enter_context(tc.tile_pool(name="ztp", bufs=1))
        zslab = ctx.enter_context(tc.tile_pool(name="zslab", bufs=1))
        osb_p = ctx.enter_context(tc.tile_pool(name="osb_p", bufs=1))
        ps = ctx.enter_context(tc.tile_pool(name="ps", bufs=1, space="PSUM"))

        # ---- constants: weights + identity ----
        wq_sb = const.tile([128, 8, 128], BF16, name="wq_sb")
        wk_sb = const.tile([128, 8, 128], BF16, name="wk_sb")
        wv_sb = const.tile([128, 8, 128], BF16, name="wv_sb")
        wo_sb = const.tile([128, 8, 128], BF16, name="wo_sb")
        for dc in range(8):
            nc.sync.dma_start(wq_sb[:, dc, :], wqT[dc * 128:(dc + 1) * 128, :])
            nc.sync.dma_start(wk_sb[:, dc, :], wkT[dc * 128:(dc + 1) * 128, :])
            nc.sync.dma_start(wv_sb[:, dc, :], wvT[dc * 128:(dc + 1) * 128, :])
            nc.sync.dma_start(wo_sb[:, dc, :], woT[dc * 128:(dc + 1) * 128, :])
        ident = const.tile([128, 128], BF16, name="ident")
        make_identity(nc, ident)

        def proj(b):
            """QKV projections for batch b -> (qt, kt, v) sbuf tiles."""
            x_c = []
            for dc in range(8):
                xc = xpool.tile([128, S], BF16, name=f"xc{dc}", tag=f"x{dc}")
                nc.gpsimd.dma_start(xc, xT[b, dc * 128:(dc + 1) * 128, :])
                x_c.append(xc)

            # weight-stationary: dc outer so each LDWEIGHTS covers 4 matmuls
            outs = []
            for w_sb, nm in ((wq_sb, "qt"), (wk_sb, "kt"), (wv_sb, "vt")):
                pool = vtpool if nm == "vt" else qk
                dest = pool.tile([128, S], BF16, name=nm, tag=nm)
                pw0 = ps.tile([128, 1024], F32, name="pw0", tag="work")
                pw1 = ps.tile([128, 1024], F32, name="pw1", tag="work")
                halves = (pw0, pw1)
                for dc in range(8):
                    for sc in range(4):
                        nc.tensor.matmul(
                            halves[sc // 2][:, (sc % 2) * 512:(sc % 2 + 1) * 512],
                            lhsT=w_sb[:, dc, :],
                            rhs=x_c[dc][:, sc * 512:(sc + 1) * 512],
                            start=(dc == 0),
                            stop=(dc == 7),
                        )
                for hh in range(2):
                    nc.vector.tensor_copy(
                        dest[:, hh * 1024:(hh + 1) * 1024], halves[hh]
                    )
                outs.append(dest)
            qt, kt, vt = outs

            # transpose VT [dh2, s] -> V [s, dh2] in 128-chunks
            v_sb = vpool.tile([128, KC, 128], BF16, name="v_sb", tag="v")
            for c in range(KC):
                pt = ps.tile([128, 128], BF16, name="pt", tag="work")
                nc.tensor.transpose(pt, vt[:, c * 128:(c + 1) * 128], ident)
                nc.vector.tensor_copy(v_sb[:, c, :], pt)
            return qt, kt, v_sb

        def scores_exp(kc, qt, kt, v_sb):
            """Scores + exp + denominators + scaled V for key-chunk kc."""
            res = []
            for h in range(2):
                hp = h * 64
                a_t = apool.tile([128, S], BF16, name=f"a{h}", tag=f"a{h}")
                accs = []
                for half in range(2):
                    pw = ps.tile([128, 1024], F32, name="psc", tag="work")
                    for qq in range(2):
                        q0 = half * 1024 + qq * 512
                        nc.tensor.matmul(
                            pw[:, qq * 512:(qq + 1) * 512],
                            lhsT=kt[hp:hp + 64, kc * 128:(kc + 1) * 128],
                            rhs=qt[hp:hp + 64, q0:q0 + 512],
                            start=True,
                            stop=True,
                        )
                    acc = small.tile([128, 1], F32, name="acc", tag="acc")
                    nc.scalar.activation(
                        a_t[:, half * 1024:(half + 1) * 1024],
                        pw,
                        EXP,
                        scale=float(SCALE),
                        accum_out=acc,
                    )
                    accs.append(acc)
                den = small.tile([128, 1], F32, name="den", tag="den")
                nc.vector.tensor_add(den, accs[0], accs[1])
                rec = small.tile([128, 1], F32, name="rec", tag="rec")
                nc.vector.reciprocal(rec, den)
                vs = small.tile([128, DH], BF16, name="vs", tag=f"vs{h}")
                nc.vector.tensor_scalar_mul(vs, v_sb[:, kc, hp:hp + 64], rec)
                res.append((a_t, vs))
            return res

        def av(kc, pair, zt_ps):
            """z^T += Vs^T @ A for key-chunk kc (both heads, col-tiled)."""
            for h in range(2):
                a_t, vs = pair[h]
                hp = h * 64
                for qc in range(4):
                    nc.tensor.matmul(
                        zt_ps[hp:hp + 64, qc * 512:(qc + 1) * 512],
                        lhsT=vs,
                        rhs=a_t[:, qc * 512:(qc + 1) * 512],
                        start=(kc == 0),
                        stop=(kc == KC - 1),
                        skip_group_check=True,
                    )

        SKEW = 3  # AV for kc issues after scores for kc+SKEW: PE never
        # stalls on the exp->denom->Vs chain (it completed ~SKEW units ago)

        def attention(b, qt, kt, v_sb):
            zt_ps = ps.tile([128, S], F32, name="zt_ps", tag="zt")
            pending = []
            for kc in range(KC):
                pending.append((kc, scores_exp(kc, qt, kt, v_sb)))
                if len(pending) > SKEW:
                    k0, pair = pending.pop(0)
                    av(k0, pair, zt_ps)
            for k0, pair in pending:
                av(k0, pair, zt_ps)

            zt_sb = ztp.tile([128, S], BF16, name="zt_sb", tag="zt_sb")
            for hh in range(2):
                nc.vector.tensor_copy(
                    zt_sb[:, hh * 1024:(hh + 1) * 1024],
                    zt_ps[:, hh * 1024:(hh + 1) * 1024],
                )
            nc.sync.dma_start(zloc[b], zt_sb)
            nc.gpsimd.collective_compute(
                "AllGather",
                mybir.AluOpType.bypass,
                replica_groups=[list(range(NCORES))],
                ins=[zloc[b].opt()],
                outs=[zfull[b][:, :].opt()],
            )

        def outproj(b):
            zf_c = []
            for j in range(4):
                zf = zslab.tile([128, 2, S], BF16, name=f"zf{j}", tag=f"zf{j}")
                nc.gpsimd.dma_start(
                    zf,
                    zfull[b][j * 256:(j + 1) * 256, :].rearrange(
                        "(c p) s -> p c s", p=128
                    ),
                )
                zf_c.append(zf)
            o_sb = osb_p.tile([128, S], F32, name="o_sb", tag="osb")
            po0 = ps.tile([128, 1024], F32, name="po0", tag="work")
            po1 = ps.tile([128, 1024], F32, name="po1", tag="work")
            halves = (po0, po1)
            for lc in range(8):
                for sc in range(4):
                    nc.tensor.matmul(
                        halves[sc // 2][:, (sc % 2) * 512:(sc % 2 + 1) * 512],
                        lhsT=wo_sb[:, lc, :],
                        rhs=zf_c[lc // 2][:, lc % 2, sc * 512:(sc + 1) * 512],
                        start=(lc == 0),
                        stop=(lc == 7),
                    )
            for hh in range(2):
                nc.vector.tensor_copy(
                    o_sb[:, hh * 1024:(hh + 1) * 1024], halves[hh]
                )
            nc.sync.dma_start(outT[:, b * S:(b + 1) * S], o_sb)

        for b in range(B):
            qt, kt, v_sb = proj(b)
            attention(b, qt, kt, v_sb)
            if b >= 1:
                outproj(b - 1)
        outproj(B - 1)


def _legalize_waits(nc):
    """This walrus build accepts only ~2 sync commands (1 wait + 1 inc) per
    instruction for the standard engine/DMA templates; Tile can emit 2-3
    waits (WAR + WAW + RAW). Hoist all but one wait of any multi-wait
    instruction onto single-wait NOPs on the same engine, immediately
    before it — the raw-bass `wait_ge; op` pattern. Drain/EventSemaphore
    templates accept many waits (the kernel-tail barrier relies on it)."""
    import bass_rust

    n = 0
    for f in nc.m.functions:
        for blk in f.blocks:
            out = []
            changed = False
            for inst in blk.instructions:
                si = inst.sync_info
                if si is not None and len(si.on_wait) > 1:
                    for w in si.on_wait[:-1]:
                        n += 1
                        out.append(
                            bass_rust.InstNoOp(
                                name=f"I-hoistwait-{n}",
                                engine=inst.engine,
                                bass_nofuse=True,
                                sync_info=bass_rust.SyncInfo(
                                    on_wait=[w], on_update=[]
                                ),
                            )
                        )
                    inst.sync_info = bass_rust.SyncInfo(
                        on_wait=[si.on_wait[-1]], on_update=list(si.on_update)
                    )
                    changed = True
                out.append(inst)
            if changed:
                blk.instructions = out


def build():
    nc = bass.Bass(
        "TRN2",
        target_bir_lowering=False,
        debug=False,
        enable_asserts=False,
        num_devices=NCORES,
    )
    xT = nc.dram_tensor("xT", [B, D, S], BF16, kind="ExternalInput").ap()
    wqT = nc.dram_tensor("wqT", [D, LPC], BF16, kind="ExternalInput").ap()
    wkT = nc.dram_tensor("wkT", [D, LPC], BF16, kind="ExternalInput").ap()
    wvT = nc.dram_tensor("wvT", [D, LPC], BF16, kind="ExternalInput").ap()
    woT = nc.dram_tensor("woT", [L, DPC], BF16, kind="ExternalInput").ap()
    outT = nc.dram_tensor("outT", [DPC, B * S], F32, kind="ExternalOutput").ap()

    with tile.TileContext(nc) as tc:
        from contextlib import ExitStack

        with ExitStack() as ctx:
            dram = ctx.enter_context(tc.tile_pool(name="dram", bufs=1, space="DRAM"))
            zloc = dram.tile([B, LPC, S], BF16, name="zloc")
            zfull = [
                dram.tile([L, S], BF16, name=f"zfull{b}", tag=f"zfull{b}",
                          addr_space="Shared")
                for b in range(B)
            ]
            _body(tc, xT, wqT, wkT, wvT, woT, outT, zloc, zfull)
    _legalize_waits(nc)
    return nc


def make_in_maps(x, Wq, Wk, Wv, Wo):
    bf = ml_dtypes.bfloat16
    x = np.asarray(x, np.float32)
    xT = np.ascontiguousarray(x.transpose(0, 2, 1)).astype(bf)  # (B, D, S)
    WoT = np.ascontiguousarray(np.asarray(Wo, np.float32).T)    # (L, D)
    in_maps = []
    for c in range(NCORES):
        rs = slice(128 * c, 128 * (c + 1))
        in_maps.append({
            "xT": xT,
            "wqT": np.ascontiguousarray(np.asarray(Wq, np.float32)[rs].T).astype(bf),
            "wkT": np.ascontiguousarray(np.asarray(Wk, np.float32)[rs].T).astype(bf),
            "wvT": np.ascontiguousarray(np.asarray(Wv, np.float32)[rs].T).astype(bf),
            "woT": np.ascontiguousarray(WoT[:, rs]).astype(bf),
        })
    return in_maps


def _install_ntff_hook_shim():
    """This container's `antenv` lacks `axon_hooks`; recreate the NTFF
    profile hook (same ctypes recipe as trn_agent_boot.trn_boot) so
    run_bass_kernel_spmd(trace=True) can capture exec_time_ns."""
    import sys
    import types
    import ctypes
    import contextlib

    try:
        import antenv.axon_hooks  # noqa: F401
        return
    except ImportError:
        pass

    hook = None
    so_path = os.environ.get("PJRT_LIBRARY_PATH")
    if so_path and os.path.exists(so_path):
        try:
            lib = ctypes.CDLL(so_path)
            if hasattr(lib, "axon_start_nrt_profile"):
                lib.axon_start_nrt_profile.argtypes = [
                    ctypes.POINTER(ctypes.c_int64),
                    ctypes.c_size_t,
                ]
                lib.axon_start_nrt_profile.restype = ctypes.c_int64
                lib.axon_stop_nrt_profile.argtypes = [ctypes.c_char_p]
                lib.axon_stop_nrt_profile.restype = ctypes.c_int64

                @contextlib.contextmanager
                def _hook(output_dir, device_ids):
                    import jax

                    jax.devices()
                    if device_ids:
                        ids = (ctypes.c_int64 * len(device_ids))(*device_ids)
                        rc = lib.axon_start_nrt_profile(ids, len(device_ids))
                    else:
                        rc = lib.axon_start_nrt_profile(None, 0)
                    if rc != 0:
                        raise RuntimeError(f"axon_start_nrt_profile rc={rc}")
                    try:
                        yield
                    finally:
                        n = lib.axon_stop_nrt_profile(str(output_dir).encode())
                        print(f"profile: {n} file(s) written to {output_dir}")

                hook = _hook
        except OSError:
            hook = None

    mod = types.ModuleType("antenv.axon_hooks")
    mod.get_axon_ntff_profile_hook = lambda: hook
    mod.set_axon_ntff_profile_hook = lambda h: None
    sys.modules["antenv.axon_hooks"] = mod
    import antenv

    antenv.axon_hooks = mod


def kernel(x, Wq, Wk, Wv, Wo):
    global LAST_EXEC_NS
    in_maps = make_in_maps(x, Wq, Wk, Wv, Wo)
    nc = build()
    trace = bool(int(os.environ.get("BASS_KERNEL_TRACE", "0")))
    if trace:
        _install_ntff_hook_shim()
    res = run_bass_kernel_spmd(nc, in_maps, core_ids=list(range(NCORES)), trace=trace)
    LAST_EXEC_NS = res.exec_time_ns
    outT = np.concatenate(
        [np.asarray(res.results[c]["outT"], np.float32) for c in range(NCORES)], axis=0
    )  # (D, B*S)
    return np.ascontiguousarray(outT.T).reshape(B, S, D).astype(np.float32)


# revision 30
# speedup vs baseline: 1.7158x; 1.6617x over previous
"""Trainium2 Bass kernel for nn_Attention_9431748182617.

Quirky attention: scores z[b,k,q] = (q_h . k_h) / sqrt(D), softmax over the
QUERY axis (per key row), out[q] = sum_k A[k,q] * v[k], then output projection.

Sharding (8 NeuronCores):
  - tensor-parallel over heads: 16 heads -> 2 heads per core.
    Each core owns rows [128c, 128c+128) of Wq/Wk/Wv (its 2 heads) and
    computes q/k/v + attention for those heads over the full batch.
  - z^T (local 128 rows of L, all of B*S) is AllGather'd per batch.
  - output projection sharded by output feature D: core c computes
    out^T rows [128c, 128c+128) using Wo^T[:, 128c:128c+128] for ALL s.
  - host concatenates the 8 out^T blocks and transposes.

All matmuls in bf16 (fp32 PSUM accumulation); exp on ScalarE in fp32 with
fused free-axis accumulation for softmax denominators; 1/denom is folded
into V rows (per-partition scalar) so no full-size normalization pass.
"""

import os

import numpy as np
import ml_dtypes

import concourse.bass as bass
import concourse.mybir as mybir
import concourse.tile as tile
from concourse.bass_utils import run_bass_kernel_spmd
from concourse.masks import make_identity

B, S, D = 4, 2048, 1024
L, H = 1024, 16
DH = L // H               # 64
NCORES = 8
LPC = L // NCORES         # 128 l-rows (= 2 heads) per core
DPC = D // NCORES         # 128 out-feature rows per core
SCALE = 1.0 / (D ** 0.5)
KC = S // 128             # 16 key chunks of 128
BF16 = mybir.dt.bfloat16
F32 = mybir.dt.float32
EXP = mybir.ActivationFunctionType.Exp

LAST_EXEC_NS = None


def _body(tc, xT, wqT, wkT, wvT, woT, outT, zloc, zfull):
    nc = tc.nc
    from contextlib import ExitStack

    with ExitStack() as ctx:
        const = ctx.enter_context(tc.tile_pool(name="const", bufs=1))
        xpool = ctx.enter_context(tc.tile_pool(name="xpool", bufs=1))
        qk = ctx.enter_context(tc.tile_pool(name="qk", bufs=2))
        vtpool = ctx.enter_context(tc.tile_pool(name="vtpool", bufs=1))
        vpool = ctx.enter_context(tc.tile_pool(name="vpool", bufs=2))
        apool = ctx.enter_context(tc.tile_pool(name="apool", bufs=7))
        small = ctx.enter_context(tc.tile_pool(name="small", bufs=8))
        ztp = ctx.enter_context(tc.tile_pool(name="ztp", bufs=2))
        zslab = ctx.enter_context(tc.tile_pool(name="zslab", bufs=1))
        osb_p = ctx.enter_context(tc.tile_pool(name="osb_p", bufs=1))
        # all 8 PSUM banks in one 4-deep [128,1024] pool: scores, AV
        # partials, projections, out-projection all cycle through it
        ps = ctx.enter_context(tc.tile_pool(name="ps", bufs=1, space="PSUM"))

        # ---- constants: weights + identity ----
        wq_sb = const.tile([128, 8, 128], BF16, name="wq_sb")
        wk_sb = const.tile([128, 8, 128], BF16, name="wk_sb")
        wv_sb = const.tile([128, 8, 128], BF16, name="wv_sb")
        wo_sb = const.tile([128, 8, 128], BF16, name="wo_sb")
        for dc in range(8):
            nc.sync.dma_start(wq_sb[:, dc, :], wqT[dc * 128:(dc + 1) * 128, :])
            nc.sync.dma_start(wk_sb[:, dc, :], wkT[dc * 128:(dc + 1) * 128, :])
            nc.sync.dma_start(wv_sb[:, dc, :], wvT[dc * 128:(dc + 1) * 128, :])
            nc.sync.dma_start(wo_sb[:, dc, :], woT[dc * 128:(dc + 1) * 128, :])
        ident = const.tile([128, 128], BF16, name="ident")
        make_identity(nc, ident)

        def proj(b):
            """QKV projections for batch b -> (qt, kt, v) sbuf tiles."""
            x_c = []
            for dc in range(8):
                xc = xpool.tile([128, S], BF16, name=f"xc{dc}", tag=f"x{dc}")
                nc.gpsimd.dma_start(xc, xT[b, dc * 128:(dc + 1) * 128, :])
                x_c.append(xc)

            # weight-stationary: dc outer so each LDWEIGHTS covers 4 matmuls
            outs = []
            for w_sb, nm in ((wq_sb, "qt"), (wk_sb, "kt"), (wv_sb, "vt")):
                pool = vtpool if nm == "vt" else qk
                dest = pool.tile([128, S], BF16, name=nm, tag=nm)
                pw0 = ps.tile([128, 1024], F32, name="pw0", tag="work", bufs=4)
                pw1 = ps.tile([128, 1024], F32, name="pw1", tag="work", bufs=4)
                halves = (pw0, pw1)
                for dc in range(8):
                    for sc in range(4):
                        nc.tensor.matmul(
                            halves[sc // 2][:, (sc % 2) * 512:(sc % 2 + 1) * 512],
                            lhsT=w_sb[:, dc, :],
                            rhs=x_c[dc][:, sc * 512:(sc + 1) * 512],
                            start=(dc == 0),
                            stop=(dc == 7),
                        )
                for hh in range(2):
                    nc.vector.tensor_copy(
                        dest[:, hh * 1024:(hh + 1) * 1024], halves[hh]
                    )
                outs.append(dest)
            qt, kt, vt = outs

            # transpose VT [dh2, s] -> V [s, dh2] in 128-chunks
            v_sb = vpool.tile([128, KC, 128], BF16, name="v_sb", tag="v")
            for c in range(KC):
                pt = ps.tile([128, 128], BF16, name="pt", tag="work", bufs=4)
                nc.tensor.transpose(pt, vt[:, c * 128:(c + 1) * 128], ident)
                nc.vector.tensor_copy(v_sb[:, c, :], pt)
            return qt, kt, v_sb

        def scores_exp(kc, qt, kt, v_sb):
            """Scores + exp + denominators + scaled V for key-chunk kc."""
            res = []
            for h in range(2):
                hp = h * 64
                a_t = apool.tile([128, S], BF16, name=f"a{h}", tag=f"a{h}")
                accs = []
                for half in range(2):
                    pw = ps.tile([128, 1024], F32, name="psc", tag="work",
                                 bufs=4)
                    for qq in range(2):
                        q0 = half * 1024 + qq * 512
                        nc.tensor.matmul(
                            pw[:, qq * 512:(qq + 1) * 512],
                            lhsT=kt[hp:hp + 64, kc * 128:(kc + 1) * 128],
                            rhs=qt[hp:hp + 64, q0:q0 + 512],
                            start=True,
                            stop=True,
                        )
                    acc = small.tile([128, 1], F32, name="acc", tag="acc")
                    nc.scalar.activation(
                        a_t[:, half * 1024:(half + 1) * 1024],
                        pw,
                        EXP,
                        scale=float(SCALE),
                        accum_out=acc,
                    )
                    accs.append(acc)
                den = small.tile([128, 1], F32, name="den", tag="den")
                nc.vector.tensor_add(den, accs[0], accs[1])
                rec = small.tile([128, 1], F32, name="rec", tag="rec")
                nc.vector.reciprocal(rec, den)
                vs = small.tile([128, DH], BF16, name="vs", tag=f"vs{h}")
                nc.vector.tensor_scalar_mul(vs, v_sb[:, kc, hp:hp + 64], rec)
                res.append((a_t, vs))
            return res

        def av_pair(units, zac, first):
            """AV for two kc units: dense 16-matmul burst into two PSUM
            tiles (accumulating over the 2 kc), then fold into the SBUF
            f32 accumulator on DVE. Keeps zT out of PSUM so the work pool
            can be 4 deep, and gives PE a long uninterrupted burst."""
            zps = [
                ps.tile([128, 1024], F32, name=f"zp{q2}", tag="work", bufs=4)
                for q2 in range(2)
            ]
            last = len(units) - 1
            for j, (kc, pair) in enumerate(units):
                for h in range(2):
                    a_t, vs = pair[h]
                    hp = h * 64
                    for qc in range(4):
                        nc.tensor.matmul(
                            zps[qc // 2][hp:hp + 64,
                                         (qc % 2) * 512:(qc % 2 + 1) * 512],
                            lhsT=vs,
                            rhs=a_t[:, qc * 512:(qc + 1) * 512],
                            start=(j == 0),
                            stop=(j == last),
                            skip_group_check=True,
                        )
            for q2 in range(2):
                sl = zac[:, q2 * 1024:(q2 + 1) * 1024]
                if first:
                    nc.vector.tensor_copy(sl, zps[q2])
                else:
                    nc.vector.tensor_add(sl, zps[q2], sl)

        def attention(b, qt, kt, v_sb):
            zac = ztp.tile([128, S], F32, name="zac", tag="zac")
            pending = []
            npairs = 0
            for kc in range(KC):
                pending.append((kc, scores_exp(kc, qt, kt, v_sb)))
                if len(pending) >= 6:
                    av_pair(pending[:2], zac, first=(npairs == 0))
                    pending = pending[2:]
                    npairs += 1
            while pending:
                av_pair(pending[:2], zac, first=(npairs == 0))
                pending = pending[2:]
                npairs += 1
            # f32 -> bf16 cast happens inside the SWDGE DMA
            nc.gpsimd.dma_start(zloc[b], zac)
            nc.gpsimd.collective_compute(
                "AllGather",
                mybir.AluOpType.bypass,
                replica_groups=[list(range(NCORES))],
                ins=[zloc[b].opt()],
                outs=[zfull[b][:, :].opt()],
            )

        def outproj(b):
            zf_c = []
            for j in range(4):
                zf = zslab.tile([128, 2, S], BF16, name=f"zf{j}", tag=f"zf{j}")
                nc.gpsimd.dma_start(
                    zf,
                    zfull[b][j * 256:(j + 1) * 256, :].rearrange(
                        "(c p) s -> p c s", p=128
                    ),
                )
                zf_c.append(zf)
            o_sb = osb_p.tile([128, S], F32, name="o_sb", tag="osb")
            po0 = ps.tile([128, 1024], F32, name="po0", tag="work", bufs=4)
            po1 = ps.tile([128, 1024], F32, name="po1", tag="work", bufs=4)
            halves = (po0, po1)
            for lc in range(8):
                for sc in range(4):
                    nc.tensor.matmul(
                        halves[sc // 2][:, (sc % 2) * 512:(sc % 2 + 1) * 512],
                        lhsT=wo_sb[:, lc, :],
                        rhs=zf_c[lc // 2][:, lc % 2, sc * 512:(sc + 1) * 512],
                        start=(lc == 0),
                        stop=(lc == 7),
                    )
            for hh in range(2):
                nc.vector.tensor_copy(
                    o_sb[:, hh * 1024:(hh + 1) * 1024], halves[hh]
                )
            nc.sync.dma_start(outT[:, b * S:(b + 1) * S], o_sb)

        for b in range(B):
            qt, kt, v_sb = proj(b)
            attention(b, qt, kt, v_sb)
            if b >= 1:
                outproj(b - 1)
        outproj(B - 1)


def _legalize_waits(nc):
    """This walrus build accepts only ~2 sync commands (1 wait + 1 inc) per
    instruction for the standard engine/DMA templates; Tile can emit 2-3
    waits (WAR + WAW + RAW). Hoist all but one wait of any multi-wait
    instruction onto single-wait NOPs on the same engine, immediately
    before it — the raw-bass `wait_ge; op` pattern. Drain/EventSemaphore
    templates accept many waits (the kernel-tail barrier relies on it)."""
    import bass_rust

    n = 0
    for f in nc.m.functions:
        for blk in f.blocks:
            out = []
            changed = False
            for inst in blk.instructions:
                si = inst.sync_info
                if si is not None and len(si.on_wait) > 1:
                    for w in si.on_wait[:-1]:
                        n += 1
                        out.append(
                            bass_rust.InstNoOp(
                                name=f"I-hoistwait-{n}",
                                engine=inst.engine,
                                bass_nofuse=True,
                                sync_info=bass_rust.SyncInfo(
                                    on_wait=[w], on_update=[]
                                ),
                            )
                        )
                    inst.sync_info = bass_rust.SyncInfo(
                        on_wait=[si.on_wait[-1]], on_update=list(si.on_update)
                    )
                    changed = True
                out.append(inst)
            if changed:
                blk.instructions = out


def build(legalize=True):
    nc = bass.Bass(
        "TRN2",
        target_bir_lowering=False,
        debug=False,
        enable_asserts=False,
        num_devices=NCORES,
    )
    xT = nc.dram_tensor("xT", [B, D, S], BF16, kind="ExternalInput").ap()
    wqT = nc.dram_tensor("wqT", [D, LPC], BF16, kind="ExternalInput").ap()
    wkT = nc.dram_tensor("wkT", [D, LPC], BF16, kind="ExternalInput").ap()
    wvT = nc.dram_tensor("wvT", [D, LPC], BF16, kind="ExternalInput").ap()
    woT = nc.dram_tensor("woT", [L, DPC], BF16, kind="ExternalInput").ap()
    outT = nc.dram_tensor("outT", [DPC, B * S], F32, kind="ExternalOutput").ap()

    with tile.TileContext(nc) as tc:
        from contextlib import ExitStack

        with ExitStack() as ctx:
            dram = ctx.enter_context(tc.tile_pool(name="dram", bufs=1, space="DRAM"))
            zloc = dram.tile([B, LPC, S], BF16, name="zloc")
            zfull = [
                dram.tile([L, S], BF16, name=f"zfull{b}", tag=f"zfull{b}",
                          addr_space="Shared")
                for b in range(B)
            ]
            _body(tc, xT, wqT, wkT, wvT, woT, outT, zloc, zfull)
    if legalize:
        # the inserted NOPs are invisible to the simulator's race-detector
        # registry; sim callers pass legalize=False (identical semantics)
        _legalize_waits(nc)
    return nc


def make_in_maps(x, Wq, Wk, Wv, Wo):
    bf = ml_dtypes.bfloat16
    x = np.asarray(x, np.float32)
    xT = np.ascontiguousarray(x.transpose(0, 2, 1)).astype(bf)  # (B, D, S)
    WoT = np.ascontiguousarray(np.asarray(Wo, np.float32).T)    # (L, D)
    in_maps = []
    for c in range(NCORES):
        rs = slice(128 * c, 128 * (c + 1))
        in_maps.append({
            "xT": xT,
            "wqT": np.ascontiguousarray(np.asarray(Wq, np.float32)[rs].T).astype(bf),
            "wkT": np.ascontiguousarray(np.asarray(Wk, np.float32)[rs].T).astype(bf),
            "wvT": np.ascontiguousarray(np.asarray(Wv, np.float32)[rs].T).astype(bf),
            "woT": np.ascontiguousarray(WoT[:, rs]).astype(bf),
        })
    return in_maps


def _install_ntff_hook_shim():
    """This container's `antenv` lacks `axon_hooks`; recreate the NTFF
    profile hook (same ctypes recipe as trn_agent_boot.trn_boot) so
    run_bass_kernel_spmd(trace=True) can capture exec_time_ns."""
    import sys
    import types
    import ctypes
    import contextlib

    try:
        import antenv.axon_hooks  # noqa: F401
        return
    except ImportError:
        pass

    hook = None
    so_path = os.environ.get("PJRT_LIBRARY_PATH")
    if so_path and os.path.exists(so_path):
        try:
            lib = ctypes.CDLL(so_path)
            if hasattr(lib, "axon_start_nrt_profile"):
                lib.axon_start_nrt_profile.argtypes = [
                    ctypes.POINTER(ctypes.c_int64),
                    ctypes.c_size_t,
                ]
                lib.axon_start_nrt_profile.restype = ctypes.c_int64
                lib.axon_stop_nrt_profile.argtypes = [ctypes.c_char_p]
                lib.axon_stop_nrt_profile.restype = ctypes.c_int64

                @contextlib.contextmanager
                def _hook(output_dir, device_ids):
                    import jax

                    jax.devices()
                    if device_ids:
                        ids = (ctypes.c_int64 * len(device_ids))(*device_ids)
                        rc = lib.axon_start_nrt_profile(ids, len(device_ids))
                    else:
                        rc = lib.axon_start_nrt_profile(None, 0)
                    if rc != 0:
                        raise RuntimeError(f"axon_start_nrt_profile rc={rc}")
                    try:
                        yield
                    finally:
                        n = lib.axon_stop_nrt_profile(str(output_dir).encode())
                        print(f"profile: {n} file(s) written to {output_dir}")

                hook = _hook
        except OSError:
            hook = None

    mod = types.ModuleType("antenv.axon_hooks")
    mod.get_axon_ntff_profile_hook = lambda: hook
    mod.set_axon_ntff_profile_hook = lambda h: None
    sys.modules["antenv.axon_hooks"] = mod
    import antenv

    antenv.axon_hooks = mod


def kernel(x, Wq, Wk, Wv, Wo):
    global LAST_EXEC_NS
    in_maps = make_in_maps(x, Wq, Wk, Wv, Wo)
    nc = build()
    trace = bool(int(os.environ.get("BASS_KERNEL_TRACE", "0")))
    if trace:
        _install_ntff_hook_shim()
    res = run_bass_kernel_spmd(nc, in_maps, core_ids=list(range(NCORES)), trace=trace)
    LAST_EXEC_NS = res.exec_time_ns
    outT = np.concatenate(
        [np.asarray(res.results[c]["outT"], np.float32) for c in range(NCORES)], axis=0
    )  # (D, B*S)
    return np.ascontiguousarray(outT.T).reshape(B, S, D).astype(np.float32)


# revision 35
# speedup vs baseline: 1.7730x; 1.0334x over previous
"""Trainium2 Bass kernel for nn_Attention_9431748182617.

Quirky attention: scores z[b,k,q] = (q_h . k_h) / sqrt(D), softmax over the
QUERY axis (per key row), out[q] = sum_k A[k,q] * v[k], then output projection.

Sharding (8 NeuronCores):
  - tensor-parallel over heads: 16 heads -> 2 heads per core.
    Each core owns rows [128c, 128c+128) of Wq/Wk/Wv (its 2 heads) and
    computes q/k/v + attention for those heads over the full batch.
  - z^T (local 128 rows of L, all of B*S) is AllGather'd per batch.
  - output projection sharded by output feature D: core c computes
    out^T rows [128c, 128c+128) using Wo^T[:, 128c:128c+128] for ALL s.
  - host concatenates the 8 out^T blocks and transposes.

All matmuls in bf16 (fp32 PSUM accumulation); exp on ScalarE in fp32 with
fused free-axis accumulation for softmax denominators; 1/denom is folded
into V rows (per-partition scalar) so no full-size normalization pass.
"""

import os

import numpy as np
import ml_dtypes

import concourse.bass as bass
import concourse.mybir as mybir
import concourse.tile as tile
from concourse.bass_utils import run_bass_kernel_spmd
from concourse.masks import make_identity

B, S, D = 4, 2048, 1024
L, H = 1024, 16
DH = L // H               # 64
NCORES = 8
LPC = L // NCORES         # 128 l-rows (= 2 heads) per core
DPC = D // NCORES         # 128 out-feature rows per core
SCALE = 1.0 / (D ** 0.5)
KC = S // 128             # 16 key chunks of 128
BF16 = mybir.dt.bfloat16
F32 = mybir.dt.float32
EXP = mybir.ActivationFunctionType.Exp

LAST_EXEC_NS = None


def _body(tc, xT, wqT, wkT, wvT, woT, outT, zloc, zfull):
    nc = tc.nc
    from contextlib import ExitStack

    with ExitStack() as ctx:
        const = ctx.enter_context(tc.tile_pool(name="const", bufs=1))
        xpool = ctx.enter_context(tc.tile_pool(name="xpool", bufs=1))
        qk = ctx.enter_context(tc.tile_pool(name="qk", bufs=2))
        vtpool = ctx.enter_context(tc.tile_pool(name="vtpool", bufs=1))
        vpool = ctx.enter_context(tc.tile_pool(name="vpool", bufs=2))
        apool = ctx.enter_context(tc.tile_pool(name="apool", bufs=7))
        small = ctx.enter_context(tc.tile_pool(name="small", bufs=8))
        ztp = ctx.enter_context(tc.tile_pool(name="ztp", bufs=2))
        zslab = ctx.enter_context(tc.tile_pool(name="zslab", bufs=1))
        osb_p = ctx.enter_context(tc.tile_pool(name="osb_p", bufs=2))
        # all 8 PSUM banks in one 4-deep [128,1024] pool: scores, AV
        # partials, projections, out-projection all cycle through it
        ps = ctx.enter_context(tc.tile_pool(name="ps", bufs=1, space="PSUM"))

        # ---- constants: weights + identity ----
        wq_sb = const.tile([128, 8, 128], BF16, name="wq_sb")
        wk_sb = const.tile([128, 8, 128], BF16, name="wk_sb")
        wv_sb = const.tile([128, 8, 128], BF16, name="wv_sb")
        wo_sb = const.tile([128, 8, 128], BF16, name="wo_sb")
        for dc in range(8):
            nc.sync.dma_start(wq_sb[:, dc, :], wqT[dc * 128:(dc + 1) * 128, :])
            nc.sync.dma_start(wk_sb[:, dc, :], wkT[dc * 128:(dc + 1) * 128, :])
            nc.sync.dma_start(wv_sb[:, dc, :], wvT[dc * 128:(dc + 1) * 128, :])
            nc.sync.dma_start(wo_sb[:, dc, :], woT[dc * 128:(dc + 1) * 128, :])
        ident = const.tile([128, 128], BF16, name="ident")
        make_identity(nc, ident)

        def load_x(b):
            x_c = []
            for dc in range(8):
                xc = xpool.tile([128, S], BF16, name=f"xc{dc}", tag=f"x{dc}")
                nc.gpsimd.dma_start(xc, xT[b, dc * 128:(dc + 1) * 128, :])
                x_c.append(xc)
            return x_c

        def proj_w(w_sb, nm, x_c):
            """One projection (Q, K or V^T layout) — weight-stationary."""
            pool = vtpool if nm == "vt" else qk
            dest = pool.tile([128, S], BF16, name=nm, tag=nm)
            for half in range(2):
                pw = ps.tile([128, 1024], F32, name="pw", tag="work", bufs=4)
                for dc in range(8):
                    for q in range(2):
                        sc = half * 2 + q
                        nc.tensor.matmul(
                            pw[:, q * 512:(q + 1) * 512],
                            lhsT=w_sb[:, dc, :],
                            rhs=x_c[dc][:, sc * 512:(sc + 1) * 512],
                            start=(dc == 0),
                            stop=(dc == 7),
                        )
                nc.vector.tensor_copy(dest[:, half * 1024:(half + 1) * 1024],
                                      pw)
            return dest

        def transpose_v(vt):
            # transpose VT [dh2, s] -> V [s, dh2] in 128-chunks
            v_sb = vpool.tile([128, KC, 128], BF16, name="v_sb", tag="v")
            for c in range(KC):
                pt = ps.tile([128, 128], BF16, name="pt", tag="work", bufs=4)
                nc.tensor.transpose(pt, vt[:, c * 128:(c + 1) * 128], ident)
                nc.vector.tensor_copy(v_sb[:, c, :], pt)
            return v_sb

        def proj(b):
            x_c = load_x(b)
            qt = proj_w(wq_sb, "qt", x_c)
            kt = proj_w(wk_sb, "kt", x_c)
            vt = proj_w(wv_sb, "vt", x_c)
            return qt, kt, transpose_v(vt)

        def scores_exp(kc, qt, kt, v_sb):
            """Scores + exp + denominators + scaled V for key-chunk kc.
            Both heads' matmuls are issued adjacently so the K=64 pairs
            co-execute in disjoint PE row-groups."""
            a_ts = [
                apool.tile([128, S], BF16, name=f"a{h}", tag=f"a{h}")
                for h in range(2)
            ]
            accs = [[], []]
            for half in range(2):
                tiles = [
                    ps.tile([128, 1024], F32, name=f"psc{h}", tag="work",
                            bufs=4)
                    for h in range(2)
                ]
                for qq in range(2):
                    q0 = half * 1024 + qq * 512
                    for h in range(2):
                        hp = h * 64
                        nc.tensor.matmul(
                            tiles[h][:, qq * 512:(qq + 1) * 512],
                            lhsT=kt[hp:hp + 64, kc * 128:(kc + 1) * 128],
                            rhs=qt[hp:hp + 64, q0:q0 + 512],
                            start=True,
                            stop=True,
                        )
                for h in range(2):
                    acc = small.tile([128, 1], F32, name="acc", tag="acc")
                    nc.scalar.activation(
                        a_ts[h][:, half * 1024:(half + 1) * 1024],
                        tiles[h],
                        EXP,
                        scale=float(SCALE),
                        accum_out=acc,
                    )
                    accs[h].append(acc)
            res = []
            for h in range(2):
                den = small.tile([128, 1], F32, name="den", tag="den")
                nc.vector.tensor_add(den, accs[h][0], accs[h][1])
                rec = small.tile([128, 1], F32, name="rec", tag="rec")
                nc.vector.reciprocal(rec, den)
                vs = small.tile([128, DH], BF16, name="vs", tag=f"vs{h}")
                nc.vector.tensor_scalar_mul(
                    vs, v_sb[:, kc, h * 64:h * 64 + 64], rec)
                res.append((a_ts[h], vs))
            return res

        def av_pair(units, zac, first):
            """AV for two kc units: dense 16-matmul burst into two PSUM
            tiles (accumulating over the 2 kc), then fold into the SBUF
            f32 accumulator on DVE. Keeps zT out of PSUM so the work pool
            can be 4 deep, and gives PE a long uninterrupted burst."""
            zps = [
                ps.tile([128, 1024], F32, name=f"zp{q2}", tag="work", bufs=4)
                for q2 in range(2)
            ]
            last = len(units) - 1
            for j, (kc, pair) in enumerate(units):
                for qc in range(4):
                    for h in range(2):
                        a_t, vs = pair[h]
                        hp = h * 64
                        nc.tensor.matmul(
                            zps[qc // 2][hp:hp + 64,
                                         (qc % 2) * 512:(qc % 2 + 1) * 512],
                            lhsT=vs,
                            rhs=a_t[:, qc * 512:(qc + 1) * 512],
                            start=(j == 0),
                            stop=(j == last),
                            skip_group_check=True,
                        )
            for q2 in range(2):
                sl = zac[:, q2 * 1024:(q2 + 1) * 1024]
                if first:
                    nc.vector.tensor_copy(sl, zps[q2])
                else:
                    nc.vector.tensor_add(sl, zps[q2], sl)

        def attention(b, cur, nxt_b):
            """Attention for batch b; the NEXT batch's x-load/projections/
            transposes are issued mid-stream so ScalarE never waits for a
            projection phase at batch boundaries. Returns next batch's
            (qt, kt, v) handles or None."""
            qt, kt, v_sb = cur
            zac = ztp.tile([128, S], F32, name="zac", tag="zac")
            pending = []
            npairs = 0
            nxt = {}
            for kc in range(KC):
                pending.append((kc, scores_exp(kc, qt, kt, v_sb)))
                if len(pending) >= 6:
                    av_pair(pending[:2], zac, first=(npairs == 0))
                    pending = pending[2:]
                    npairs += 1
                if nxt_b is not None:
                    if kc == 4:
                        nxt["x"] = load_x(nxt_b)
                    elif kc == 8:
                        nxt["qt"] = proj_w(wq_sb, "qt", nxt["x"])
                    elif kc == 10:
                        nxt["kt"] = proj_w(wk_sb, "kt", nxt["x"])
                    elif kc == 12:
                        nxt["vt"] = proj_w(wv_sb, "vt", nxt["x"])
                    elif kc == 14:
                        nxt["v"] = transpose_v(nxt["vt"])
            while pending:
                av_pair(pending[:2], zac, first=(npairs == 0))
                pending = pending[2:]
                npairs += 1
            # flush + AllGather per s-half (smaller exposed tail; the
            # f32 -> bf16 cast happens inside the SWDGE DMA)
            for half in range(2):
                nc.gpsimd.dma_start(
                    zloc[b, half], zac[:, half * 1024:(half + 1) * 1024])
                nc.gpsimd.collective_compute(
                    "AllGather",
                    mybir.AluOpType.bypass,
                    replica_groups=[list(range(NCORES))],
                    ins=[zloc[b, half].opt()],
                    outs=[zfull[2 * b + half][:, :].opt()],
                )
            return (nxt["qt"], nxt["kt"], nxt["v"]) if nxt_b is not None \
                else None

        def outproj(b):
            for half in range(2):
                zf_c = []
                for j in range(4):
                    zf = zslab.tile([128, 2, S // 2], BF16, name=f"zf{j}",
                                    tag=f"zf{j}")
                    nc.gpsimd.dma_start(
                        zf,
                        zfull[2 * b + half][j * 256:(j + 1) * 256, :]
                        .rearrange("(c p) s -> p c s", p=128),
                    )
                    zf_c.append(zf)
                po = ps.tile([128, 1024], F32, name="po", tag="work", bufs=4)
                for lc in range(8):
                    for sc in range(2):
                        nc.tensor.matmul(
                            po[:, sc * 512:(sc + 1) * 512],
                            lhsT=wo_sb[:, lc, :],
                            rhs=zf_c[lc // 2][:, lc % 2,
                                              sc * 512:(sc + 1) * 512],
                            start=(lc == 0),
                            stop=(lc == 7),
                        )
                o_sb = osb_p.tile([128, S // 2], F32, name="o_sb", tag="osb")
                nc.vector.tensor_copy(o_sb, po)
                nc.sync.dma_start(
                    outT[:, b * S + half * 1024:b * S + (half + 1) * 1024],
                    o_sb)

        cur = proj(0)
        for b in range(B):
            cur = attention(b, cur, b + 1 if b < B - 1 else None)
            if b >= 1:
                outproj(b - 1)
        outproj(B - 1)


def _legalize_waits(nc):
    """This walrus build accepts only ~2 sync commands (1 wait + 1 inc) per
    instruction for the standard engine/DMA templates; Tile can emit 2-3
    waits (WAR + WAW + RAW). Hoist all but one wait of any multi-wait
    instruction onto single-wait NOPs on the same engine, immediately
    before it — the raw-bass `wait_ge; op` pattern. Drain/EventSemaphore
    templates accept many waits (the kernel-tail barrier relies on it)."""
    import bass_rust

    n = 0
    for f in nc.m.functions:
        for blk in f.blocks:
            out = []
            changed = False
            for inst in blk.instructions:
                si = inst.sync_info
                if si is not None and len(si.on_wait) > 1:
                    for w in si.on_wait[:-1]:
                        n += 1
                        out.append(
                            bass_rust.InstNoOp(
                                name=f"I-hoistwait-{n}",
                                engine=inst.engine,
                                bass_nofuse=True,
                                sync_info=bass_rust.SyncInfo(
                                    on_wait=[w], on_update=[]
                                ),
                            )
                        )
                    inst.sync_info = bass_rust.SyncInfo(
                        on_wait=[si.on_wait[-1]], on_update=list(si.on_update)
                    )
                    changed = True
                out.append(inst)
            if changed:
                blk.instructions = out


def build(legalize=True):
    nc = bass.Bass(
        "TRN2",
        target_bir_lowering=False,
        debug=False,
        enable_asserts=False,
        num_devices=NCORES,
    )
    xT = nc.dram_tensor("xT", [B, D, S], BF16, kind="ExternalInput").ap()
    wqT = nc.dram_tensor("wqT", [D, LPC], BF16, kind="ExternalInput").ap()
    wkT = nc.dram_tensor("wkT", [D, LPC], BF16, kind="ExternalInput").ap()
    wvT = nc.dram_tensor("wvT", [D, LPC], BF16, kind="ExternalInput").ap()
    woT = nc.dram_tensor("woT", [L, DPC], BF16, kind="ExternalInput").ap()
    outT = nc.dram_tensor("outT", [DPC, B * S], F32, kind="ExternalOutput").ap()

    with tile.TileContext(nc) as tc:
        from contextlib import ExitStack

        with ExitStack() as ctx:
            dram = ctx.enter_context(tc.tile_pool(name="dram", bufs=1, space="DRAM"))
            zloc = dram.tile([B, 2, LPC, S // 2], BF16, name="zloc")
            zfull = [
                dram.tile([L, S // 2], BF16, name=f"zfull{i}", tag=f"zfull{i}",
                          addr_space="Shared")
                for i in range(2 * B)
            ]
            _body(tc, xT, wqT, wkT, wvT, woT, outT, zloc, zfull)
    if legalize:
        # the inserted NOPs are invisible to the simulator's race-detector
        # registry; sim callers pass legalize=False (identical semantics)
        _legalize_waits(nc)
    return nc


def make_in_maps(x, Wq, Wk, Wv, Wo):
    bf = ml_dtypes.bfloat16
    x = np.asarray(x, np.float32)
    xT = np.ascontiguousarray(x.transpose(0, 2, 1)).astype(bf)  # (B, D, S)
    WoT = np.ascontiguousarray(np.asarray(Wo, np.float32).T)    # (L, D)
    in_maps = []
    for c in range(NCORES):
        rs = slice(128 * c, 128 * (c + 1))
        in_maps.append({
            "xT": xT,
            "wqT": np.ascontiguousarray(np.asarray(Wq, np.float32)[rs].T).astype(bf),
            "wkT": np.ascontiguousarray(np.asarray(Wk, np.float32)[rs].T).astype(bf),
            "wvT": np.ascontiguousarray(np.asarray(Wv, np.float32)[rs].T).astype(bf),
            "woT": np.ascontiguousarray(WoT[:, rs]).astype(bf),
        })
    return in_maps


def _install_ntff_hook_shim():
    """This container's `antenv` lacks `axon_hooks`; recreate the NTFF
    profile hook (same ctypes recipe as trn_agent_boot.trn_boot) so
    run_bass_kernel_spmd(trace=True) can capture exec_time_ns."""
    import sys
    import types
    import ctypes
    import contextlib

    try:
        import antenv.axon_hooks  # noqa: F401
        return
    except ImportError:
        pass

    hook = None
    so_path = os.environ.get("PJRT_LIBRARY_PATH")
    if so_path and os.path.exists(so_path):
        try:
            lib = ctypes.CDLL(so_path)
            if hasattr(lib, "axon_start_nrt_profile"):
                lib.axon_start_nrt_profile.argtypes = [
                    ctypes.POINTER(ctypes.c_int64),
                    ctypes.c_size_t,
                ]
                lib.axon_start_nrt_profile.restype = ctypes.c_int64
                lib.axon_stop_nrt_profile.argtypes = [ctypes.c_char_p]
                lib.axon_stop_nrt_profile.restype = ctypes.c_int64

                @contextlib.contextmanager
                def _hook(output_dir, device_ids):
                    import jax

                    jax.devices()
                    if device_ids:
                        ids = (ctypes.c_int64 * len(device_ids))(*device_ids)
                        rc = lib.axon_start_nrt_profile(ids, len(device_ids))
                    else:
                        rc = lib.axon_start_nrt_profile(None, 0)
                    if rc != 0:
                        raise RuntimeError(f"axon_start_nrt_profile rc={rc}")
                    try:
                        yield
                    finally:
                        n = lib.axon_stop_nrt_profile(str(output_dir).encode())
                        print(f"profile: {n} file(s) written to {output_dir}")

                hook = _hook
        except OSError:
            hook = None

    mod = types.ModuleType("antenv.axon_hooks")
    mod.get_axon_ntff_profile_hook = lambda: hook
    mod.set_axon_ntff_profile_hook = lambda h: None
    sys.modules["antenv.axon_hooks"] = mod
    import antenv

    antenv.axon_hooks = mod


def kernel(x, Wq, Wk, Wv, Wo):
    global LAST_EXEC_NS
    in_maps = make_in_maps(x, Wq, Wk, Wv, Wo)
    nc = build()
    trace = bool(int(os.environ.get("BASS_KERNEL_TRACE", "0")))
    if trace:
        _install_ntff_hook_shim()
    res = run_bass_kernel_spmd(nc, in_maps, core_ids=list(range(NCORES)), trace=trace)
    LAST_EXEC_NS = res.exec_time_ns
    outT = np.concatenate(
        [np.asarray(res.results[c]["outT"], np.float32) for c in range(NCORES)], axis=0
    )  # (D, B*S)
    return np.ascontiguousarray(outT.T).reshape(B, S, D).astype(np.float32)
